# revision 24
# baseline (speedup 1.0000x reference)
"""Trainium2 Bass kernel for nn_Cross_MultiAttention_Q_masked.

Full-input contract: kernel(**inputs) takes the complete arrays from
setup_inputs() and returns (out, att_weights) like the reference.

Sharding: 8 cores = batch (4) x query-half (2). Each core computes all 8
heads for its 1024 queries, processed as two 512-query passes to fit SBUF.

Dataflow per core:
  QKV loaded via SWDGE cast-DMA (f32 -> bf16), transposed through the PE
  with regular identity matmuls, projected per head in transposed layout
  (qTe/kTe [65, h, sq] with a packed ones/kv-mask contraction row).
  Scores (K=65 bf16 matmul, fp32 PSUM) -> m1 = (raw/8)*A on DVE/ACT ->
  exp on ACT (row-sum via accum_out) -> Ez = E - b (DVE fp16 2x) ->
  w = Ez * recip(denom) on GpSimd -> att_weights stored via SWDGE
  cast-DMA (fp16 -> f32). w transposed via identity matmuls, att and the
  output projection on the PE.

Numerics: bf16 matmul path, fp16 elementwise path, fp32 accumulation.
- kv_mask folded as an additive -8e9 row in the packed K=65 score matmul.
- binary mask folded as A = aff*(1-b) pre-exp plus a post-exp subtract:
  Ez = exp((raw/8)*A) - b, exact at masked positions.
- softmax denominator = sum(exp) - sum(b); no max pass needed (scores are
  bounded; masked lanes are exactly exp(0)=1 pre-subtraction).
"""

import contextlib

import numpy as np

import concourse.bass as bass
import concourse.mybir as mybir
import concourse.tile as tile
from concourse.tile import add_dep_helper
from concourse.masks import make_identity

FP32 = mybir.dt.float32
BF16 = mybir.dt.bfloat16
F16 = mybir.dt.float16
U8 = mybir.dt.uint8
AF = mybir.ActivationFunctionType
OP = mybir.AluOpType

B, SQ, SKV = 4, 2048, 1024
DQ, DKV, DOUT = 512, 768, 256
H, DH = 8, 64
SQC = SQ // 2          # queries per core
NSQT = SQC // 128      # 8 query tiles per core
NHALF = 2
NTH = NSQT // NHALF    # 4 query tiles per half-pass
SQH = SQC // NHALF     # 512 queries per half-pass
NKP = SKV // 128       # 8 kv tiles
NKQ = DQ // 128        # 4 k-tiles for Wq
NKKV = DKV // 128      # 6 k-tiles for Wk/Wv
NEGK = -8.0e9          # additive kv-mask value, pre-/8 scale


def _build() -> bass.Bass:
    nc = bass.Bass()

    Qh = nc.dram_tensor("Qh", [SQC, DQ], FP32, kind="ExternalInput")
    Kf = nc.dram_tensor("Kf", [SKV, DKV], FP32, kind="ExternalInput")
    Vf = nc.dram_tensor("Vf", [SKV, DKV], FP32, kind="ExternalInput")
    aff = nc.dram_tensor("aff", [SQC, SKV], FP32, kind="ExternalInput")
    binm = nc.dram_tensor("binm", [SQC, SKV], U8, kind="ExternalInput")
    kvm = nc.dram_tensor("kvm", [1, SKV], U8, kind="ExternalInput")
    Wq_d = nc.dram_tensor("Wq", [DQ, DQ], FP32, kind="ExternalInput")
    Wk_d = nc.dram_tensor("Wk", [DKV, DQ], FP32, kind="ExternalInput")
    Wv_d = nc.dram_tensor("Wv", [DKV, DQ], FP32, kind="ExternalInput")
    Wo_d = nc.dram_tensor("Wo", [DQ, DOUT], FP32, kind="ExternalInput")
    bq_d = nc.dram_tensor("bq", [1, DQ], FP32, kind="ExternalInput")
    bk_d = nc.dram_tensor("bk", [1, DQ], FP32, kind="ExternalInput")
    bv_d = nc.dram_tensor("bv", [1, DQ], FP32, kind="ExternalInput")
    bo_d = nc.dram_tensor("bo", [1, DOUT], FP32, kind="ExternalInput")
    w_out = nc.dram_tensor("w", [H, SQC, SKV], FP32, kind="ExternalOutput")
    out_d = nc.dram_tensor("out", [SQC, DOUT], FP32, kind="ExternalOutput")

    sync_dmas: list = []
    gp_dmas: list = []
    tail_extra: list = []
    eng_tail: dict = {"tensor": [], "vector": [], "scalar": [], "gpsimd": []}

    def sdma(out, in_):
        i = nc.sync.dma_start(out=out, in_=in_)
        sync_dmas.append(i)
        return i

    def gdma(out, in_):
        i = nc.gpsimd.dma_start(out=out, in_=in_)
        gp_dmas.append(i)
        return i

    with tile.TileContext(nc) as tc, contextlib.ExitStack() as ctx:
        const = ctx.enter_context(tc.tile_pool(name="const", bufs=1))
        small = ctx.enter_context(tc.tile_pool(name="small", bufs=2))
        ps_s = ctx.enter_context(tc.tile_pool(name="ps_s", bufs=2, space="PSUM"))
        ps_m = ctx.enter_context(tc.tile_pool(name="ps_m", bufs=2, space="PSUM"))
        ps_t = ctx.enter_context(tc.tile_pool(name="ps_t", bufs=2, space="PSUM"))

        # ---- persistent tiles --------------------------------------------
        ones16 = const.tile([1, SKV], F16)
        I16 = const.tile([128, 128], F16)
        Ibf = const.tile([128, 128], BF16)
        Wo16 = const.tile([64, H, DOUT], F16)    # [p, h, n] = Wo[64h+p, n]
        bo16 = const.tile([1, DOUT], F16)
        qTe = const.tile([65, H, SQC], BF16)
        kTe = const.tile([65, H, SQC], BF16)
        v16 = const.tile([128, NKP, DQ], F16)
        attT = const.tile([64, H, SQC], F16)
        zeros16 = const.tile([128, SKV], F16)
        bsum = const.tile([128, NSQT], FP32)     # cols NTH*half + s

        nc.vector.memset(zeros16[:], 0.0)
        nc.vector.memset(ones16[:], 1.0)
        nc.vector.memset(qTe[64:65, :, :], 1.0)

        def ident(ap):
            nc.gpsimd.memset(ap, 0.0)
            i = nc.gpsimd.affine_select(
                out=ap, in_=ap, compare_op=OP.not_equal, fill=1.0, base=0,
                pattern=[[-1, ap.shape[0]]], channel_multiplier=1,
            )
            eng_tail["gpsimd"].append(i)
            return i

        ident(I16[:])
        ident(Ibf[:])

        # ---- phase 1: weights, QKV transpose, projections -----------------
        with tc.tile_pool(name="stage", bufs=1) as stage:
            ones_bf = stage.tile([1, SKV], BF16, tag="onesb")
            nc.vector.memset(ones_bf[:], 1.0)
            Wq_bf = stage.tile([128, NKQ, DQ], BF16, tag="wq")
            Wk_bf = stage.tile([128, NKKV, DQ], BF16, tag="wk")
            Wv_bf = stage.tile([128, NKKV, DQ], BF16, tag="wv")
            bq_bf = stage.tile([1, DQ], BF16, tag="bq")
            bk_bf = stage.tile([1, DQ], BF16, tag="bk")
            bv_bf = stage.tile([1, DQ], BF16, tag="bv")
            kvrow = stage.tile([1, SKV], BF16, tag="kvr")

            gdma(Wq_bf[:], Wq_d[:].rearrange("(t p) m -> p t m", p=128))
            gdma(Wk_bf[:], Wk_d[:].rearrange("(t p) m -> p t m", p=128))
            gdma(Wv_bf[:], Wv_d[:].rearrange("(t p) m -> p t m", p=128))
            gdma(Wo16[:], Wo_d[:].rearrange("(h p) n -> p h n", p=64))
            gdma(bq_bf[:], bq_d[:])
            gdma(bk_bf[:], bk_d[:])
            gdma(bv_bf[:], bv_d[:])
            gdma(bo16[:], bo_d[:])
            kv_u8 = stage.tile([1, SKV], U8, tag="kv_u8")
            sdma(kv_u8[:], kvm[:])
            nc.vector.tensor_scalar(kvrow[:], kv_u8[:], float(NEGK), None, OP.mult)

            def load_and_transpose(src_dram, ncols_t, T_bf):
                # src [S, D] f32 -> bf16 (cast-DMA) -> T_bf [128, D/128, S]
                s_sb = stage.tile([128, NSQT, DKV], BF16, tag="stg_in")
                d = src_dram.shape[1]
                gdma(s_sb[:, :, 0:d], src_dram[:].rearrange("(s p) d -> p s d", p=128))
                for j in range(ncols_t):
                    for g in range(2):  # groups of 4 source tiles
                        psT = ps_t.tile([128, 512], FP32, tag="ps_t")
                        for si in range(4 * g, 4 * g + 4):
                            nc.tensor.matmul(
                                psT[:, 128 * (si % 4) : 128 * (si % 4) + 128],
                                s_sb[:, si, 128 * j : 128 * j + 128],
                                Ibf[:],
                                start=True,
                                stop=True,
                            )
                        if (j + g) % 2 == 0:
                            nc.vector.tensor_copy(
                                T_bf[:, j, 512 * g : 512 * g + 512], psT[:]
                            )
                        else:
                            nc.scalar.copy(
                                T_bf[:, j, 512 * g : 512 * g + 512], psT[:]
                            )

            def proj_T(T_bf, W_bf, bias_bf, dst, nk):
                # per-head M=64 projection into transposed layout
                for h in range(H):
                    psQ = ps_s.tile([128, SQC], FP32, tag="ps_s")
                    for n2 in range(2):
                        cs = slice(512 * n2, 512 * n2 + 512)
                        for k in range(nk):
                            nc.tensor.matmul(
                                psQ[0:64, cs],
                                W_bf[:, k, 64 * h : 64 * h + 64],
                                T_bf[:, k, cs],
                                start=(k == 0),
                                stop=False,
                            )
                        nc.tensor.matmul(
                            psQ[0:64, cs],
                            bias_bf[0:1, 64 * h : 64 * h + 64],
                            ones_bf[0:1, 0:512],
                            start=False,
                            stop=True,
                        )
                    if h % 2 == 0:
                        nc.vector.tensor_copy(dst[0:64, h, :], psQ[0:64, :])
                    else:
                        nc.scalar.copy(dst[0:64, h, :], psQ[0:64, :])

            QT_bf = stage.tile([128, NKKV, SQC], BF16, tag="stg_T")
            load_and_transpose(Qh, NKQ, QT_bf)
            proj_T(QT_bf, Wq_bf, bq_bf, qTe, NKQ)

            KT_bf = stage.tile([128, NKKV, SQC], BF16, tag="stg_T")
            load_and_transpose(Kf, NKKV, KT_bf)
            proj_T(KT_bf, Wk_bf, bk_bf, kTe, NKKV)
            for h in range(H):
                nc.vector.tensor_copy(kTe[64:65, h, :], kvrow[0:1, :])

            VT_bf = stage.tile([128, NKKV, SQC], BF16, tag="stg_T")
            load_and_transpose(Vf, NKKV, VT_bf)
            for mt in range(NKP):
                psV = ps_s.tile([128, SQC], FP32, tag="ps_s")
                for k in range(NKKV):
                    nc.tensor.matmul(
                        psV[:, 0:512],
                        VT_bf[:, k, 128 * mt : 128 * mt + 128],
                        Wv_bf[:, k, 0:512],
                        start=(k == 0),
                        stop=False,
                    )
                nc.tensor.matmul(
                    psV[:, 0:512],
                    ones_bf[0:1, 0:128],
                    bv_bf[0:1, 0:512],
                    start=False,
                    stop=True,
                )
                if mt % 2 == 0:
                    nc.vector.tensor_copy(v16[:, mt, :], psV[:, 0:512])
                else:
                    nc.scalar.copy(v16[:, mt, :], psV[:, 0:512])

        # ---- phase 2: attention over two query half-passes ----------------
        with tc.tile_pool(name="mask", bufs=2) as maskp, tc.tile_pool(
            name="big", bufs=4
        ) as big:
            for half in range(NHALF):
                rows = slice(half * SQH, (half + 1) * SQH)
                A0 = maskp.tile([128, NTH, SKV], F16, tag="A0")
                b16 = maskp.tile([128, NTH, SKV], F16, tag="b16")
                b_u8 = maskp.tile([128, NTH, SKV], U8, tag="b_u8")
                gdma(A0[:], aff[rows, :].rearrange("(s p) k -> p s k", p=128))
                gdma(b16[:], binm[rows, :].rearrange("(s p) k -> p s k", p=128))
                sdma(b_u8[:], binm[rows, :].rearrange("(s p) k -> p s k", p=128))
                for s in range(NTH):
                    nc.vector.copy_predicated(A0[:, s, :], b_u8[:, s, :], zeros16[:])
                    nc.vector.tensor_reduce(
                        bsum[:, NTH * half + s : NTH * half + s + 1],
                        b16[:, s, :],
                        mybir.AxisListType.X,
                        OP.add,
                    )

                for h in range(H):
                    m1 = big.tile([128, NTH, SKV], F16, tag="big")
                    stats = small.tile([128, 3 * NTH], FP32, tag="stats")
                    esum = stats[:, 0:NTH]
                    denom = stats[:, NTH : 2 * NTH]
                    recip = stats[:, 2 * NTH : 3 * NTH]
                    for s in range(NTH):
                        sqt = NTH * half + s
                        ps = ps_s.tile([128, SKV], FP32, tag="ps_s")
                        qsl = qTe[0:65, h, 128 * sqt : 128 * sqt + 128]
                        nc.tensor.matmul(
                            ps[:, 0:512], qsl, kTe[0:65, h, 0:512],
                            start=True, stop=True,
                        )
                        nc.tensor.matmul(
                            ps[:, 512:1024], qsl, kTe[0:65, h, 512:1024],
                            start=True, stop=True,
                        )
                        if (h * NTH + s) % 2 == 0:
                            nc.vector.scalar_tensor_tensor(
                                m1[:, s, :], ps[:], 0.125, A0[:, s, :],
                                OP.mult, OP.mult,
                            )
                        else:
                            # m0 must be bf16: kv-masked scores are -1e9,
                            # far outside fp16 range (inf*0 -> NaN in the
                            # multiply below if m0 were fp16).
                            m0 = big.tile([128, SKV], BF16, tag="m0")
                            nc.scalar.mul(m0[:], ps[:], 0.125)
                            nc.vector.tensor_tensor(
                                m1[:, s, :], m0[:], A0[:, s, :], OP.mult
                            )
                    E = big.tile([128, NTH, SKV], F16, tag="big")
                    for s in range(NTH):
                        nc.scalar.activation(
                            E[:, s, :], m1[:, s, :], AF.Exp,
                            accum_out=esum[:, s : s + 1],
                        )
                    Ez = big.tile([128, NTH, SKV], F16, tag="big")
                    nc.vector.tensor_tensor(
                        Ez[:].rearrange("p a b -> p (a b)"),
                        E[:].rearrange("p a b -> p (a b)"),
                        b16[:].rearrange("p a b -> p (a b)"),
                        OP.subtract,
                    )
                    nc.vector.tensor_tensor(
                        denom, esum, bsum[:, NTH * half : NTH * half + NTH],
                        OP.subtract,
                    )
                    nc.vector.reciprocal(recip, denom)
                    w16 = big.tile([128, NTH, SKV], F16, tag="big")
                    for s in range(NTH):
                        i_ts = nc.gpsimd.tensor_scalar(
                            w16[:, s, :], Ez[:, s, :],
                            recip[:, s : s + 1], None, OP.mult,
                        )
                        if h == H - 1:
                            eng_tail["gpsimd"].append(i_ts)
                    # att_weights out: fp16 -> f32 cast during SWDGE DMA
                    gdma(
                        w_out[h, rows, :].rearrange("(s p) k -> p s k", p=128),
                        w16[:],
                    )

                    # wT via identity matmuls (out = w_tile.T @ I)
                    wT = big.tile([128, NKP, SQH], F16, tag="bigT")
                    for kp in range(NKP):
                        psT = ps_t.tile([128, 512], FP32, tag="ps_t")
                        for s in range(NTH):
                            nc.tensor.matmul(
                                psT[:, 128 * s : 128 * s + 128],
                                w16[:, s, 128 * kp : 128 * kp + 128],
                                I16[:],
                                start=True,
                                stop=True,
                            )
                        if kp % 2 == 0:
                            nc.vector.tensor_copy(wT[:, kp, :], psT[:])
                        else:
                            i_sc = nc.scalar.copy(wT[:, kp, :], psT[:])
                            if h == H - 1:
                                eng_tail["scalar"].append(i_sc)
                    psA = ps_m.tile([128, 512], FP32, tag="ps_m")
                    for kp in range(NKP):
                        nc.tensor.matmul(
                            psA[0:64, 0:SQH],
                            v16[:, kp, 64 * h : 64 * h + 64],
                            wT[:, kp, :],
                            start=(kp == 0),
                            stop=(kp == NKP - 1),
                        )
                    i_at = nc.vector.tensor_copy(
                        attT[0:64, h, half * SQH : half * SQH + SQH],
                        psA[0:64, 0:SQH],
                    )
                    if h == H - 1:
                        tail_extra.append(i_at)

            # ---- phase 3: output projection ------------------------------
            for sqt in range(NSQT):
                psO = ps_m.tile([128, 512], FP32, tag="ps_m")
                for h in range(H):
                    nc.tensor.matmul(
                        psO[:, 0:256],
                        attT[0:64, h, 128 * sqt : 128 * sqt + 128],
                        Wo16[0:64, h, 0:256],
                        start=(h == 0),
                        stop=False,
                    )
                i_mm = nc.tensor.matmul(
                    psO[:, 0:256], ones16[0:1, 0:128], bo16[0:1, :],
                    start=False, stop=True,
                )
                if sqt >= NSQT - 2:
                    eng_tail["tensor"].append(i_mm)
                outt = small.tile([128, DOUT], FP32, tag="outsb")
                i_oc = nc.vector.tensor_copy(outt[:], psO[:, 0:256])
                if sqt >= NSQT - 2:
                    tail_extra.append(i_oc)
                    eng_tail["vector"].append(i_oc)
                sdma(out_d[128 * sqt : 128 * sqt + 128, :], outt[:])

            # ---- sync collapse (1 wait per CTRL instruction limit) --------
            deps = (
                [i for lst in eng_tail.values() for i in lst]
                + gp_dmas[-10:]
                + sync_dmas[-12:]
                + tail_extra
            )
            for dep in deps:
                nop = nc.sync.nop()
                add_dep_helper(nop.ins, dep.ins, sync=True, reason="collapse")

    _split_multiwaits(nc)
    return nc


def _split_multiwaits(nc: bass.Bass) -> None:
    """This walrus build allows only ONE sync wait per instruction on the
    CTRL / DMA / TensorScalarPtr pseudo paths. Waits are checked by the
    issuing sequencer in program order, so hoisting all-but-one wait onto
    injected same-engine NoOps immediately before the instruction is
    equivalent."""
    ctr = 0
    for fn in nc.m.functions:
        for blk in fn.blocks:
            new_insts = []
            for inst in blk.instructions:
                si = inst.sync_info
                if si is not None and si.on_wait and len(si.on_wait) > 1:
                    waits = list(si.on_wait)
                    for w in waits[:-1]:
                        ctr += 1
                        nop = mybir.InstNoOp(name=f"I-wsplit-{ctr}", ins=[], outs=[])
                        nop.engine = inst.engine
                        nop.sync_info = mybir.SyncInfo(on_wait=[w], on_update=[])
                        nc.inst_map[nop.name] = nop
                        new_insts.append(nop)
                    inst.sync_info = mybir.SyncInfo(
                        on_wait=[waits[-1]], on_update=list(si.on_update)
                    )
                new_insts.append(inst)
            blk.instructions = new_insts


_NC_CACHE: list = []


def _get_nc() -> bass.Bass:
    if not _NC_CACHE:
        _NC_CACHE.append(_build())
    return _NC_CACHE[0]


def kernel(**inputs) -> tuple:
    from concourse.bass_utils import run_bass_kernel_spmd

    nc = _get_nc()
    f32 = np.float32
    Q = np.ascontiguousarray(np.asarray(inputs["Q"], f32))
    K = np.ascontiguousarray(np.asarray(inputs["K"], f32))
    V = np.ascontiguousarray(np.asarray(inputs["V"], f32))
    aff = np.ascontiguousarray(np.asarray(inputs["q_aff_mask"], f32))
    binm = np.ascontiguousarray(np.asarray(inputs["q_binary_mask"])).view(np.uint8)
    kvm = np.ascontiguousarray(np.asarray(inputs["kv_mask"])).view(np.uint8)
    reps = {
        "Wq": np.ascontiguousarray(np.asarray(inputs["Wq"], f32)),
        "Wk": np.ascontiguousarray(np.asarray(inputs["Wk"], f32)),
        "Wv": np.ascontiguousarray(np.asarray(inputs["Wv"], f32)),
        "Wo": np.ascontiguousarray(np.asarray(inputs["Wo"], f32)),
        "bq": np.asarray(inputs["bq"], f32).reshape(1, DQ),
        "bk": np.asarray(inputs["bk"], f32).reshape(1, DQ),
        "bv": np.asarray(inputs["bv"], f32).reshape(1, DQ),
        "bo": np.asarray(inputs["bo"], f32).reshape(1, DOUT),
    }
    in_maps = []
    for c in range(8):
        b, half = c // 2, c % 2
        sl = slice(half * SQC, (half + 1) * SQC)
        in_maps.append(
            {
                "Qh": Q[b, sl],
                "Kf": K[b],
                "Vf": V[b],
                "aff": aff[b, sl],
                "binm": binm[b, sl],
                "kvm": kvm[b].reshape(1, SKV),
                **reps,
            }
        )
    global _last_in_maps
    _last_in_maps = in_maps
    res = run_bass_kernel_spmd(nc, in_maps, core_ids=list(range(8)))
    out = np.zeros((B, SQ, DOUT), f32)
    att_w = np.zeros((B, H, SQ, SKV), f32)
    for c in range(8):
        b, half = c // 2, c % 2
        sl = slice(half * SQC, (half + 1) * SQC)
        out[b, sl] = res.results[c]["out"]
        att_w[b, :, sl, :] = res.results[c]["w"]
    return out, att_w


if __name__ == "__main__":
    nc = _build()
    print(f"build OK: {len(nc.inst_map)} instructions")


# revision 25
# speedup vs baseline: 3.1160x; 3.1160x over previous
"""Trainium2 Bass kernel for nn_Cross_MultiAttention_Q_masked.

Full-input contract: kernel(**inputs) takes the complete arrays from
setup_inputs() and returns (out, att_weights) like the reference.

Sharding: 8 cores = batch (4) x query-half (2). Each core computes all 8
heads for its 1024 queries, processed as two 512-query passes to fit SBUF.

Dataflow per core:
  QKV loaded via SWDGE cast-DMA (f32 -> bf16), transposed through the PE
  with regular identity matmuls, projected per head in transposed layout
  (qTe/kTe [65, h, sq] with a packed ones/kv-mask contraction row).
  Scores (K=65 bf16 matmul, fp32 PSUM) -> m1 = (raw/8)*A on DVE/ACT ->
  exp on ACT (row-sum via accum_out) -> Ez = E - b (DVE fp16 2x) ->
  w = Ez * recip(denom) on GpSimd -> att_weights stored via SWDGE
  cast-DMA (fp16 -> f32). w transposed via identity matmuls, att and the
  output projection on the PE.

Numerics: bf16 matmul path, fp16 elementwise path, fp32 accumulation.
- kv_mask folded as an additive -8e9 row in the packed K=65 score matmul.
- binary mask folded as A = aff*(1-b) pre-exp plus a post-exp subtract:
  Ez = exp((raw/8)*A) - b, exact at masked positions.
- softmax denominator = sum(exp) - sum(b); no max pass needed (scores are
  bounded; masked lanes are exactly exp(0)=1 pre-subtraction).
"""

import contextlib

import numpy as np

import concourse.bass as bass
import concourse.mybir as mybir
import concourse.tile as tile
from concourse.tile import add_dep_helper
from concourse.masks import make_identity

FP32 = mybir.dt.float32
BF16 = mybir.dt.bfloat16
F16 = mybir.dt.float16
U8 = mybir.dt.uint8
AF = mybir.ActivationFunctionType
OP = mybir.AluOpType

B, SQ, SKV = 4, 2048, 1024
DQ, DKV, DOUT = 512, 768, 256
H, DH = 8, 64
SQC = SQ // 2          # queries per core
NSQT = SQC // 128      # 8 query tiles per core
NHALF = 2
NTH = NSQT // NHALF    # 4 query tiles per half-pass
SQH = SQC // NHALF     # 512 queries per half-pass
NKP = SKV // 128       # 8 kv tiles
NKQ = DQ // 128        # 4 k-tiles for Wq
NKKV = DKV // 128      # 6 k-tiles for Wk/Wv
NEGK = -8.0e9          # additive kv-mask value, pre-/8 scale


def _build() -> bass.Bass:
    nc = bass.Bass()

    Qh = nc.dram_tensor("Qh", [SQC, DQ], FP32, kind="ExternalInput")
    Kf = nc.dram_tensor("Kf", [SKV, DKV], FP32, kind="ExternalInput")
    Vf = nc.dram_tensor("Vf", [SKV, DKV], FP32, kind="ExternalInput")
    aff = nc.dram_tensor("aff", [SQC, SKV], FP32, kind="ExternalInput")
    binm = nc.dram_tensor("binm", [SQC, SKV], U8, kind="ExternalInput")
    kvm = nc.dram_tensor("kvm", [1, SKV], U8, kind="ExternalInput")
    Wq_d = nc.dram_tensor("Wq", [DQ, DQ], FP32, kind="ExternalInput")
    Wk_d = nc.dram_tensor("Wk", [DKV, DQ], FP32, kind="ExternalInput")
    Wv_d = nc.dram_tensor("Wv", [DKV, DQ], FP32, kind="ExternalInput")
    Wo_d = nc.dram_tensor("Wo", [DQ, DOUT], FP32, kind="ExternalInput")
    bq_d = nc.dram_tensor("bq", [1, DQ], FP32, kind="ExternalInput")
    bk_d = nc.dram_tensor("bk", [1, DQ], FP32, kind="ExternalInput")
    bv_d = nc.dram_tensor("bv", [1, DQ], FP32, kind="ExternalInput")
    bo_d = nc.dram_tensor("bo", [1, DOUT], FP32, kind="ExternalInput")
    w_out = nc.dram_tensor("w", [H, SQC, SKV], FP32, kind="ExternalOutput")
    out_d = nc.dram_tensor("out", [SQC, DOUT], FP32, kind="ExternalOutput")

    sync_dmas: list = []
    gp_dmas: list = []
    tail_extra: list = []
    eng_tail: dict = {"tensor": [], "vector": [], "scalar": [], "gpsimd": []}

    def sdma(out, in_):
        i = nc.sync.dma_start(out=out, in_=in_)
        sync_dmas.append(i)
        return i

    def gdma(out, in_):
        i = nc.gpsimd.dma_start(out=out, in_=in_)
        gp_dmas.append(i)
        return i

    with tile.TileContext(nc) as tc, contextlib.ExitStack() as ctx:
        const = ctx.enter_context(tc.tile_pool(name="const", bufs=1))
        small = ctx.enter_context(tc.tile_pool(name="small", bufs=2))
        ps_s = ctx.enter_context(tc.tile_pool(name="ps_s", bufs=2, space="PSUM"))
        ps_m = ctx.enter_context(tc.tile_pool(name="ps_m", bufs=2, space="PSUM"))
        ps_t = ctx.enter_context(tc.tile_pool(name="ps_t", bufs=2, space="PSUM"))

        # ---- persistent tiles --------------------------------------------
        ones16 = const.tile([1, SKV], F16)
        I16 = const.tile([128, 128], F16)
        Ibf = const.tile([128, 128], BF16)
        Wo16 = const.tile([64, H, DOUT], F16)    # [p, h, n] = Wo[64h+p, n]
        bo16 = const.tile([1, DOUT], F16)
        qTe = const.tile([65, H, SQC], BF16)
        kTe = const.tile([65, H, SQC], BF16)
        v16 = const.tile([128, NKP, DQ], F16)
        attT = const.tile([64, H, SQC], F16)
        zeros16 = const.tile([128, SKV], F16)
        bsum = const.tile([128, NSQT], FP32)     # cols NTH*half + s

        nc.vector.memset(zeros16[:], 0.0)
        nc.vector.memset(ones16[:], 1.0)
        nc.vector.memset(qTe[64:65, :, :], 1.0)

        def ident(ap):
            nc.gpsimd.memset(ap, 0.0)
            i = nc.gpsimd.affine_select(
                out=ap, in_=ap, compare_op=OP.not_equal, fill=1.0, base=0,
                pattern=[[-1, ap.shape[0]]], channel_multiplier=1,
            )
            eng_tail["gpsimd"].append(i)
            return i

        ident(I16[:])
        ident(Ibf[:])

        # ---- phase 1: weights, QKV transpose, projections -----------------
        with tc.tile_pool(name="stage", bufs=1) as stage:
            ones_bf = stage.tile([1, SKV], BF16, tag="onesb")
            nc.vector.memset(ones_bf[:], 1.0)
            Wq_bf = stage.tile([128, NKQ, DQ], BF16, tag="wq")
            Wk_bf = stage.tile([128, NKKV, DQ], BF16, tag="wk")
            Wv_bf = stage.tile([128, NKKV, DQ], BF16, tag="wv")
            bq_bf = stage.tile([1, DQ], BF16, tag="bq")
            bk_bf = stage.tile([1, DQ], BF16, tag="bk")
            bv_bf = stage.tile([1, DQ], BF16, tag="bv")
            kvrow = stage.tile([1, SKV], BF16, tag="kvr")

            gdma(Wq_bf[:], Wq_d[:].rearrange("(t p) m -> p t m", p=128))
            gdma(Wk_bf[:], Wk_d[:].rearrange("(t p) m -> p t m", p=128))
            gdma(Wv_bf[:], Wv_d[:].rearrange("(t p) m -> p t m", p=128))
            gdma(Wo16[:], Wo_d[:].rearrange("(h p) n -> p h n", p=64))
            gdma(bq_bf[:], bq_d[:])
            gdma(bk_bf[:], bk_d[:])
            gdma(bv_bf[:], bv_d[:])
            gdma(bo16[:], bo_d[:])
            kv_u8 = stage.tile([1, SKV], U8, tag="kv_u8")
            sdma(kv_u8[:], kvm[:])
            nc.vector.tensor_scalar(kvrow[:], kv_u8[:], float(NEGK), None, OP.mult)

            def load_and_transpose(src_dram, ncols_t, T_bf):
                # src [S, D] f32 -> bf16 (cast-DMA) -> T_bf [128, D/128, S]
                s_sb = stage.tile([128, NSQT, DKV], BF16, tag="stg_in")
                d = src_dram.shape[1]
                gdma(s_sb[:, :, 0:d], src_dram[:].rearrange("(s p) d -> p s d", p=128))
                for j in range(ncols_t):
                    for g in range(2):  # groups of 4 source tiles
                        psT = ps_t.tile([128, 512], FP32, tag="ps_t")
                        for si in range(4 * g, 4 * g + 4):
                            nc.tensor.matmul(
                                psT[:, 128 * (si % 4) : 128 * (si % 4) + 128],
                                s_sb[:, si, 128 * j : 128 * j + 128],
                                Ibf[:],
                                start=True,
                                stop=True,
                            )
                        if (j + g) % 2 == 0:
                            nc.vector.tensor_copy(
                                T_bf[:, j, 512 * g : 512 * g + 512], psT[:]
                            )
                        else:
                            nc.scalar.copy(
                                T_bf[:, j, 512 * g : 512 * g + 512], psT[:]
                            )

            def proj_T(T_bf, W_bf, bias_bf, dst, nk):
                # per-head M=64 projection into transposed layout
                for h in range(H):
                    psQ = ps_s.tile([128, SQC], FP32, tag="ps_s")
                    for n2 in range(2):
                        cs = slice(512 * n2, 512 * n2 + 512)
                        for k in range(nk):
                            nc.tensor.matmul(
                                psQ[0:64, cs],
                                W_bf[:, k, 64 * h : 64 * h + 64],
                                T_bf[:, k, cs],
                                start=(k == 0),
                                stop=False,
                            )
                        nc.tensor.matmul(
                            psQ[0:64, cs],
                            bias_bf[0:1, 64 * h : 64 * h + 64],
                            ones_bf[0:1, 0:512],
                            start=False,
                            stop=True,
                        )
                    if h % 2 == 0:
                        nc.vector.tensor_copy(dst[0:64, h, :], psQ[0:64, :])
                    else:
                        nc.scalar.copy(dst[0:64, h, :], psQ[0:64, :])

            QT_bf = stage.tile([128, NKKV, SQC], BF16, tag="stg_T")
            load_and_transpose(Qh, NKQ, QT_bf)
            proj_T(QT_bf, Wq_bf, bq_bf, qTe, NKQ)

            KT_bf = stage.tile([128, NKKV, SQC], BF16, tag="stg_T")
            load_and_transpose(Kf, NKKV, KT_bf)
            proj_T(KT_bf, Wk_bf, bk_bf, kTe, NKKV)
            for h in range(H):
                nc.vector.tensor_copy(kTe[64:65, h, :], kvrow[0:1, :])

            VT_bf = stage.tile([128, NKKV, SQC], BF16, tag="stg_T")
            load_and_transpose(Vf, NKKV, VT_bf)
            for mt in range(NKP):
                psV = ps_s.tile([128, SQC], FP32, tag="ps_s")
                for k in range(NKKV):
                    nc.tensor.matmul(
                        psV[:, 0:512],
                        VT_bf[:, k, 128 * mt : 128 * mt + 128],
                        Wv_bf[:, k, 0:512],
                        start=(k == 0),
                        stop=False,
                    )
                nc.tensor.matmul(
                    psV[:, 0:512],
                    ones_bf[0:1, 0:128],
                    bv_bf[0:1, 0:512],
                    start=False,
                    stop=True,
                )
                if mt % 2 == 0:
                    nc.vector.tensor_copy(v16[:, mt, :], psV[:, 0:512])
                else:
                    nc.scalar.copy(v16[:, mt, :], psV[:, 0:512])

        # ---- phase 2: attention over two query half-passes ----------------
        with tc.tile_pool(name="mask", bufs=2) as maskp, tc.tile_pool(
            name="big", bufs=4
        ) as big:
            for half in range(NHALF):
                rows = slice(half * SQH, (half + 1) * SQH)
                A0 = maskp.tile([128, NTH, SKV], F16, tag="A0")
                b16 = maskp.tile([128, NTH, SKV], F16, tag="b16")
                b_u8 = maskp.tile([128, NTH, SKV], U8, tag="b_u8")
                gdma(A0[:], aff[rows, :].rearrange("(s p) k -> p s k", p=128))
                gdma(b16[:], binm[rows, :].rearrange("(s p) k -> p s k", p=128))
                sdma(b_u8[:], binm[rows, :].rearrange("(s p) k -> p s k", p=128))
                for s in range(NTH):
                    nc.vector.copy_predicated(A0[:, s, :], b_u8[:, s, :], zeros16[:])
                    nc.vector.tensor_reduce(
                        bsum[:, NTH * half + s : NTH * half + s + 1],
                        b16[:, s, :],
                        mybir.AxisListType.X,
                        OP.add,
                    )

                for h in range(H):
                    m1 = big.tile([128, NTH, SKV], F16, tag="big")
                    stats = small.tile([128, 3 * NTH], FP32, tag="stats")
                    esum = stats[:, 0:NTH]
                    denom = stats[:, NTH : 2 * NTH]
                    recip = stats[:, 2 * NTH : 3 * NTH]
                    for s in range(NTH):
                        sqt = NTH * half + s
                        ps = ps_s.tile([128, SKV], FP32, tag="ps_s")
                        qsl = qTe[0:65, h, 128 * sqt : 128 * sqt + 128]
                        nc.tensor.matmul(
                            ps[:, 0:512], qsl, kTe[0:65, h, 0:512],
                            start=True, stop=True,
                        )
                        nc.tensor.matmul(
                            ps[:, 512:1024], qsl, kTe[0:65, h, 512:1024],
                            start=True, stop=True,
                        )
                        if (h * NTH + s) % 2 == 0:
                            nc.vector.scalar_tensor_tensor(
                                m1[:, s, :], ps[:], 0.125, A0[:, s, :],
                                OP.mult, OP.mult,
                            )
                        else:
                            # m0 must be bf16: kv-masked scores are -1e9,
                            # far outside fp16 range (inf*0 -> NaN in the
                            # multiply below if m0 were fp16).
                            m0 = big.tile([128, SKV], BF16, tag="m0")
                            nc.scalar.mul(m0[:], ps[:], 0.125)
                            nc.vector.tensor_tensor(
                                m1[:, s, :], m0[:], A0[:, s, :], OP.mult
                            )
                    E = big.tile([128, NTH, SKV], F16, tag="big")
                    for s in range(NTH):
                        nc.scalar.activation(
                            E[:, s, :], m1[:, s, :], AF.Exp,
                            accum_out=esum[:, s : s + 1],
                        )
                    Ez = big.tile([128, NTH, SKV], F16, tag="big")
                    nc.vector.tensor_tensor(
                        Ez[:].rearrange("p a b -> p (a b)"),
                        E[:].rearrange("p a b -> p (a b)"),
                        b16[:].rearrange("p a b -> p (a b)"),
                        OP.subtract,
                    )
                    nc.vector.tensor_tensor(
                        denom, esum, bsum[:, NTH * half : NTH * half + NTH],
                        OP.subtract,
                    )
                    nc.vector.reciprocal(recip, denom)
                    w16 = big.tile([128, NTH, SKV], F16, tag="big")
                    for s in range(NTH):
                        nc.vector.tensor_scalar(
                            w16[:, s, :], Ez[:, s, :],
                            recip[:, s : s + 1], None, OP.mult,
                        )
                    # att_weights out: fp16 -> f32 cast during SWDGE DMA
                    gdma(
                        w_out[h, rows, :].rearrange("(s p) k -> p s k", p=128),
                        w16[:],
                    )

                    # wT via identity matmuls (out = w_tile.T @ I)
                    wT = big.tile([128, NKP, SQH], F16, tag="bigT")
                    for kp in range(NKP):
                        psT = ps_t.tile([128, 512], FP32, tag="ps_t")
                        for s in range(NTH):
                            nc.tensor.matmul(
                                psT[:, 128 * s : 128 * s + 128],
                                w16[:, s, 128 * kp : 128 * kp + 128],
                                I16[:],
                                start=True,
                                stop=True,
                            )
                        if kp % 2 == 0:
                            nc.vector.tensor_copy(wT[:, kp, :], psT[:])
                        else:
                            i_sc = nc.scalar.copy(wT[:, kp, :], psT[:])
                            if h == H - 1:
                                eng_tail["scalar"].append(i_sc)
                    psA = ps_m.tile([128, 512], FP32, tag="ps_m")
                    for kp in range(NKP):
                        nc.tensor.matmul(
                            psA[0:64, 0:SQH],
                            v16[:, kp, 64 * h : 64 * h + 64],
                            wT[:, kp, :],
                            start=(kp == 0),
                            stop=(kp == NKP - 1),
                        )
                    i_at = nc.vector.tensor_copy(
                        attT[0:64, h, half * SQH : half * SQH + SQH],
                        psA[0:64, 0:SQH],
                    )
                    if h == H - 1:
                        tail_extra.append(i_at)

            # ---- phase 3: output projection ------------------------------
            for sqt in range(NSQT):
                psO = ps_m.tile([128, 512], FP32, tag="ps_m")
                for h in range(H):
                    nc.tensor.matmul(
                        psO[:, 0:256],
                        attT[0:64, h, 128 * sqt : 128 * sqt + 128],
                        Wo16[0:64, h, 0:256],
                        start=(h == 0),
                        stop=False,
                    )
                i_mm = nc.tensor.matmul(
                    psO[:, 0:256], ones16[0:1, 0:128], bo16[0:1, :],
                    start=False, stop=True,
                )
                if sqt >= NSQT - 2:
                    eng_tail["tensor"].append(i_mm)
                outt = small.tile([128, DOUT], FP32, tag="outsb")
                i_oc = nc.vector.tensor_copy(outt[:], psO[:, 0:256])
                if sqt >= NSQT - 2:
                    tail_extra.append(i_oc)
                    eng_tail["vector"].append(i_oc)
                sdma(out_d[128 * sqt : 128 * sqt + 128, :], outt[:])

            # ---- sync collapse (1 wait per CTRL instruction limit) --------
            deps = (
                [i for lst in eng_tail.values() for i in lst]
                + gp_dmas[-10:]
                + sync_dmas[-12:]
                + tail_extra
            )
            for dep in deps:
                nop = nc.sync.nop()
                add_dep_helper(nop.ins, dep.ins, sync=True, reason="collapse")

    _split_multiwaits(nc)
    return nc


def _split_multiwaits(nc: bass.Bass) -> None:
    """This walrus build allows only ONE sync wait per instruction on the
    CTRL / DMA / TensorScalarPtr pseudo paths. Waits are checked by the
    issuing sequencer in program order, so hoisting all-but-one wait onto
    injected same-engine NoOps immediately before the instruction is
    equivalent."""
    ctr = 0
    for fn in nc.m.functions:
        for blk in fn.blocks:
            new_insts = []
            for inst in blk.instructions:
                si = inst.sync_info
                if si is not None and si.on_wait and len(si.on_wait) > 1:
                    waits = list(si.on_wait)
                    for w in waits[:-1]:
                        ctr += 1
                        nop = mybir.InstNoOp(name=f"I-wsplit-{ctr}", ins=[], outs=[])
                        nop.engine = inst.engine
                        nop.sync_info = mybir.SyncInfo(on_wait=[w], on_update=[])
                        nc.inst_map[nop.name] = nop
                        new_insts.append(nop)
                    inst.sync_info = mybir.SyncInfo(
                        on_wait=[waits[-1]], on_update=list(si.on_update)
                    )
                new_insts.append(inst)
            blk.instructions = new_insts


_NC_CACHE: list = []


def _get_nc() -> bass.Bass:
    if not _NC_CACHE:
        _NC_CACHE.append(_build())
    return _NC_CACHE[0]


def kernel(**inputs) -> tuple:
    from concourse.bass_utils import run_bass_kernel_spmd

    nc = _get_nc()
    f32 = np.float32
    Q = np.ascontiguousarray(np.asarray(inputs["Q"], f32))
    K = np.ascontiguousarray(np.asarray(inputs["K"], f32))
    V = np.ascontiguousarray(np.asarray(inputs["V"], f32))
    aff = np.ascontiguousarray(np.asarray(inputs["q_aff_mask"], f32))
    binm = np.ascontiguousarray(np.asarray(inputs["q_binary_mask"])).view(np.uint8)
    kvm = np.ascontiguousarray(np.asarray(inputs["kv_mask"])).view(np.uint8)
    reps = {
        "Wq": np.ascontiguousarray(np.asarray(inputs["Wq"], f32)),
        "Wk": np.ascontiguousarray(np.asarray(inputs["Wk"], f32)),
        "Wv": np.ascontiguousarray(np.asarray(inputs["Wv"], f32)),
        "Wo": np.ascontiguousarray(np.asarray(inputs["Wo"], f32)),
        "bq": np.asarray(inputs["bq"], f32).reshape(1, DQ),
        "bk": np.asarray(inputs["bk"], f32).reshape(1, DQ),
        "bv": np.asarray(inputs["bv"], f32).reshape(1, DQ),
        "bo": np.asarray(inputs["bo"], f32).reshape(1, DOUT),
    }
    in_maps = []
    for c in range(8):
        b, half = c // 2, c % 2
        sl = slice(half * SQC, (half + 1) * SQC)
        in_maps.append(
            {
                "Qh": Q[b, sl],
                "Kf": K[b],
                "Vf": V[b],
                "aff": aff[b, sl],
                "binm": binm[b, sl],
                "kvm": kvm[b].reshape(1, SKV),
                **reps,
            }
        )
    global _last_in_maps
    _last_in_maps = in_maps
    res = run_bass_kernel_spmd(nc, in_maps, core_ids=list(range(8)))
    out = np.zeros((B, SQ, DOUT), f32)
    att_w = np.zeros((B, H, SQ, SKV), f32)
    for c in range(8):
        b, half = c // 2, c % 2
        sl = slice(half * SQC, (half + 1) * SQC)
        out[b, sl] = res.results[c]["out"]
        att_w[b, :, sl, :] = res.results[c]["w"]
    return out, att_w


if __name__ == "__main__":
    nc = _build()
    print(f"build OK: {len(nc.inst_map)} instructions")


# revision 30
# speedup vs baseline: 3.1939x; 1.0250x over previous
"""Trainium2 Bass kernel for nn_Cross_MultiAttention_Q_masked.

Full-input contract: kernel(**inputs) takes the complete arrays from
setup_inputs() and returns (out, att_weights) like the reference.

Sharding: 8 cores = batch (4) x query-half (2). Each core computes all 8
heads for its 1024 queries, processed as two 512-query passes to fit SBUF.

Dataflow per core:
  QKV loaded via SWDGE cast-DMA (f32 -> bf16), transposed through the PE
  with regular identity matmuls, projected per head in transposed layout
  (qTe/kTe [65, h, sq] with a packed ones/kv-mask contraction row).
  Scores (K=65 bf16 matmul, fp32 PSUM) -> m1 = (raw/8)*A on DVE/ACT ->
  exp on ACT (row-sum via accum_out) -> Ez = E - b (DVE fp16 2x) ->
  w = Ez * recip(denom) on GpSimd -> att_weights stored via SWDGE
  cast-DMA (fp16 -> f32). w transposed via identity matmuls, att and the
  output projection on the PE.

Numerics: bf16 matmul path, fp16 elementwise path, fp32 accumulation.
- kv_mask folded as an additive -8e9 row in the packed K=65 score matmul.
- binary mask folded as A = aff*(1-b) pre-exp plus a post-exp subtract:
  Ez = exp((raw/8)*A) - b, exact at masked positions.
- softmax denominator = sum(exp) - sum(b); no max pass needed (scores are
  bounded; masked lanes are exactly exp(0)=1 pre-subtraction).
"""

import contextlib

import numpy as np

import concourse.bass as bass
import concourse.mybir as mybir
import concourse.tile as tile
from concourse.tile import add_dep_helper
from concourse.masks import make_identity

FP32 = mybir.dt.float32
BF16 = mybir.dt.bfloat16
F16 = mybir.dt.float16
U8 = mybir.dt.uint8
AF = mybir.ActivationFunctionType
OP = mybir.AluOpType

B, SQ, SKV = 4, 2048, 1024
DQ, DKV, DOUT = 512, 768, 256
H, DH = 8, 64
SQC = SQ // 2          # queries per core
NSQT = SQC // 128      # 8 query tiles per core
NHALF = 2
NTH = NSQT // NHALF    # 4 query tiles per half-pass
SQH = SQC // NHALF     # 512 queries per half-pass
NKP = SKV // 128       # 8 kv tiles
NKQ = DQ // 128        # 4 k-tiles for Wq
NKKV = DKV // 128      # 6 k-tiles for Wk/Wv
NEGK = -8.0e9          # additive kv-mask value, pre-/8 scale


def _build() -> bass.Bass:
    nc = bass.Bass(num_swdge_queues=2)

    Qh = nc.dram_tensor("Qh", [SQC, DQ], FP32, kind="ExternalInput")
    Kf = nc.dram_tensor("Kf", [SKV, DKV], FP32, kind="ExternalInput")
    Vf = nc.dram_tensor("Vf", [SKV, DKV], FP32, kind="ExternalInput")
    aff = nc.dram_tensor("aff", [SQC, SKV], FP32, kind="ExternalInput")
    binm = nc.dram_tensor("binm", [SQC, SKV], U8, kind="ExternalInput")
    kvm = nc.dram_tensor("kvm", [1, SKV], U8, kind="ExternalInput")
    Wq_d = nc.dram_tensor("Wq", [DQ, DQ], FP32, kind="ExternalInput")
    Wk_d = nc.dram_tensor("Wk", [DKV, DQ], FP32, kind="ExternalInput")
    Wv_d = nc.dram_tensor("Wv", [DKV, DQ], FP32, kind="ExternalInput")
    Wo_d = nc.dram_tensor("Wo", [DQ, DOUT], FP32, kind="ExternalInput")
    bq_d = nc.dram_tensor("bq", [1, DQ], FP32, kind="ExternalInput")
    bk_d = nc.dram_tensor("bk", [1, DQ], FP32, kind="ExternalInput")
    bv_d = nc.dram_tensor("bv", [1, DQ], FP32, kind="ExternalInput")
    bo_d = nc.dram_tensor("bo", [1, DOUT], FP32, kind="ExternalInput")
    w_out = nc.dram_tensor("w", [H, SQC, SKV], FP32, kind="ExternalOutput")
    out_d = nc.dram_tensor("out", [SQC, DOUT], FP32, kind="ExternalOutput")

    sync_dmas: list = []
    gp_dmas: list = []
    tail_extra: list = []
    eng_tail: dict = {"tensor": [], "vector": [], "scalar": [], "gpsimd": []}

    def sdma(out, in_):
        i = nc.sync.dma_start(out=out, in_=in_)
        sync_dmas.append(i)
        return i

    def gdma(out, in_):
        i = nc.gpsimd.dma_start(out=out, in_=in_)
        gp_dmas.append(i)
        return i

    with tile.TileContext(nc) as tc, contextlib.ExitStack() as ctx:
        const = ctx.enter_context(tc.tile_pool(name="const", bufs=1))
        small = ctx.enter_context(tc.tile_pool(name="small", bufs=2))
        ps_s = ctx.enter_context(tc.tile_pool(name="ps_s", bufs=2, space="PSUM"))
        ps_m = ctx.enter_context(tc.tile_pool(name="ps_m", bufs=1, space="PSUM"))
        ps_t = ctx.enter_context(tc.tile_pool(name="ps_t", bufs=3, space="PSUM"))

        # ---- persistent tiles --------------------------------------------
        ones16 = const.tile([1, SKV], F16)
        I16 = const.tile([128, 128], F16)
        Ibf = const.tile([128, 128], BF16)
        Wo16 = const.tile([64, H, DOUT], F16)    # [p, h, n] = Wo[64h+p, n]
        bo16 = const.tile([1, DOUT], F16)
        qTe = const.tile([65, H, SQC], BF16)
        kTe = const.tile([65, H, SQC], BF16)
        v16 = const.tile([128, NKP, DQ], F16)
        attT = const.tile([64, H, SQC], F16)
        zeros16 = const.tile([128, SKV], F16)
        bsum = const.tile([128, NSQT], FP32)     # cols NTH*half + s

        nc.vector.memset(zeros16[:], 0.0)
        nc.vector.memset(ones16[:], 1.0)
        nc.vector.memset(qTe[64:65, :, :], 1.0)

        def ident(ap):
            nc.gpsimd.memset(ap, 0.0)
            i = nc.gpsimd.affine_select(
                out=ap, in_=ap, compare_op=OP.not_equal, fill=1.0, base=0,
                pattern=[[-1, ap.shape[0]]], channel_multiplier=1,
            )
            eng_tail["gpsimd"].append(i)
            return i

        ident(I16[:])
        ident(Ibf[:])

        # ---- phase 1: weights, QKV transpose, projections -----------------
        with tc.tile_pool(name="stage", bufs=1) as stage:
            ones_bf = stage.tile([1, SKV], BF16, tag="onesb")
            nc.vector.memset(ones_bf[:], 1.0)
            Wq_bf = stage.tile([128, NKQ, DQ], BF16, tag="wq")
            Wk_bf = stage.tile([128, NKKV, DQ], BF16, tag="wk")
            Wv_bf = stage.tile([128, NKKV, DQ], BF16, tag="wv")
            bq_bf = stage.tile([1, DQ], BF16, tag="bq")
            bk_bf = stage.tile([1, DQ], BF16, tag="bk")
            bv_bf = stage.tile([1, DQ], BF16, tag="bv")
            kvrow = stage.tile([1, SKV], BF16, tag="kvr")

            gdma(Wq_bf[:], Wq_d[:].rearrange("(t p) m -> p t m", p=128))
            gdma(Wk_bf[:], Wk_d[:].rearrange("(t p) m -> p t m", p=128))
            gdma(Wv_bf[:], Wv_d[:].rearrange("(t p) m -> p t m", p=128))
            gdma(Wo16[:], Wo_d[:].rearrange("(h p) n -> p h n", p=64))
            gdma(bq_bf[:], bq_d[:])
            gdma(bk_bf[:], bk_d[:])
            gdma(bv_bf[:], bv_d[:])
            gdma(bo16[:], bo_d[:])
            kv_u8 = stage.tile([1, SKV], U8, tag="kv_u8")
            sdma(kv_u8[:], kvm[:])
            nc.vector.tensor_scalar(kvrow[:], kv_u8[:], float(NEGK), None, OP.mult)

            def load_and_transpose(src_dram, ncols_t, T_bf):
                # src [S, D] f32 -> bf16 (cast-DMA) -> T_bf [128, D/128, S]
                s_sb = stage.tile([128, NSQT, DKV], BF16, tag="stg_in")
                d = src_dram.shape[1]
                gdma(s_sb[:, :, 0:d], src_dram[:].rearrange("(s p) d -> p s d", p=128))
                for j in range(ncols_t):
                    for g in range(2):  # groups of 4 source tiles
                        psT = ps_t.tile([128, 512], FP32, tag="ps_t")
                        for si in range(4 * g, 4 * g + 4):
                            nc.tensor.matmul(
                                psT[:, 128 * (si % 4) : 128 * (si % 4) + 128],
                                s_sb[:, si, 128 * j : 128 * j + 128],
                                Ibf[:],
                                start=True,
                                stop=True,
                            )
                        if (j + g) % 2 == 0:
                            nc.vector.tensor_copy(
                                T_bf[:, j, 512 * g : 512 * g + 512], psT[:]
                            )
                        else:
                            nc.scalar.copy(
                                T_bf[:, j, 512 * g : 512 * g + 512], psT[:]
                            )

            def proj_T(T_bf, W_bf, bias_bf, dst, nk):
                # per-head M=64 projection into transposed layout
                for h in range(H):
                    psQ = ps_s.tile([128, SQC], FP32, tag="ps_s")
                    for n2 in range(2):
                        cs = slice(512 * n2, 512 * n2 + 512)
                        for k in range(nk):
                            nc.tensor.matmul(
                                psQ[0:64, cs],
                                W_bf[:, k, 64 * h : 64 * h + 64],
                                T_bf[:, k, cs],
                                start=(k == 0),
                                stop=False,
                            )
                        nc.tensor.matmul(
                            psQ[0:64, cs],
                            bias_bf[0:1, 64 * h : 64 * h + 64],
                            ones_bf[0:1, 0:512],
                            start=False,
                            stop=True,
                        )
                    if h % 2 == 0:
                        nc.vector.tensor_copy(dst[0:64, h, :], psQ[0:64, :])
                    else:
                        nc.scalar.copy(dst[0:64, h, :], psQ[0:64, :])

            QT_bf = stage.tile([128, NKKV, SQC], BF16, tag="stg_T")
            load_and_transpose(Qh, NKQ, QT_bf)
            proj_T(QT_bf, Wq_bf, bq_bf, qTe, NKQ)

            KT_bf = stage.tile([128, NKKV, SQC], BF16, tag="stg_T")
            load_and_transpose(Kf, NKKV, KT_bf)
            proj_T(KT_bf, Wk_bf, bk_bf, kTe, NKKV)
            for h in range(H):
                nc.vector.tensor_copy(kTe[64:65, h, :], kvrow[0:1, :])

            VT_bf = stage.tile([128, NKKV, SQC], BF16, tag="stg_T")
            load_and_transpose(Vf, NKKV, VT_bf)
            for mt in range(NKP):
                psV = ps_s.tile([128, SQC], FP32, tag="ps_s")
                for k in range(NKKV):
                    nc.tensor.matmul(
                        psV[:, 0:512],
                        VT_bf[:, k, 128 * mt : 128 * mt + 128],
                        Wv_bf[:, k, 0:512],
                        start=(k == 0),
                        stop=False,
                    )
                nc.tensor.matmul(
                    psV[:, 0:512],
                    ones_bf[0:1, 0:128],
                    bv_bf[0:1, 0:512],
                    start=False,
                    stop=True,
                )
                if mt % 2 == 0:
                    nc.vector.tensor_copy(v16[:, mt, :], psV[:, 0:512])
                else:
                    nc.scalar.copy(v16[:, mt, :], psV[:, 0:512])

        # ---- phase 2: attention over two query half-passes ----------------
        with tc.tile_pool(name="mask", bufs=1) as maskp, tc.tile_pool(
            name="big", bufs=1
        ) as big:
            for half in range(NHALF):
                rows = slice(half * SQH, (half + 1) * SQH)
                A0 = maskp.tile([128, NTH, SKV], F16, tag="A0")
                b16 = maskp.tile([128, NTH, SKV], F16, tag="b16")
                b_u8 = maskp.tile([128, NTH, SKV], U8, tag="b_u8")
                attTh = maskp.tile([64, H, SQH], F16, tag="attTh")
                sdma(b_u8[:], binm[rows, :].rearrange("(s p) k -> p s k", p=128))
                nc.vector.tensor_copy(
                    b16[:].rearrange("p a b -> p (a b)"),
                    b_u8[:].rearrange("p a b -> p (a b)"),
                )
                for s in range(NTH):
                    r0 = half * SQH + 128 * s
                    affc = maskp.tile([128, SKV], FP32, tag="affc", bufs=2)
                    sdma(affc[:], aff[r0 : r0 + 128, :])
                    nc.vector.tensor_copy(A0[:, s, :], affc[:])
                    nc.vector.copy_predicated(A0[:, s, :], b_u8[:, s, :], zeros16[:])
                    nc.vector.tensor_reduce(
                        bsum[:, NTH * half + s : NTH * half + s + 1],
                        b16[:, s, :],
                        mybir.AxisListType.X,
                        OP.add,
                    )

                for h in range(H):
                    m1 = big.tile([128, NTH, SKV], F16, tag="big", bufs=5)
                    stats = small.tile([128, 3 * NTH], FP32, tag="stats")
                    esum = stats[:, 0:NTH]
                    denom = stats[:, NTH : 2 * NTH]
                    recip = stats[:, 2 * NTH : 3 * NTH]
                    for s in range(NTH):
                        sqt = NTH * half + s
                        ps = ps_s.tile([128, SKV], FP32, tag="ps_s")
                        qsl = qTe[0:65, h, 128 * sqt : 128 * sqt + 128]
                        nc.tensor.matmul(
                            ps[:, 0:512], qsl, kTe[0:65, h, 0:512],
                            start=True, stop=True,
                        )
                        nc.tensor.matmul(
                            ps[:, 512:1024], qsl, kTe[0:65, h, 512:1024],
                            start=True, stop=True,
                        )
                        if (h * NTH + s) % 2 == 0:
                            nc.vector.scalar_tensor_tensor(
                                m1[:, s, :], ps[:], 0.125, A0[:, s, :],
                                OP.mult, OP.mult,
                            )
                        else:
                            # m0 must be bf16: kv-masked scores are -1e9,
                            # far outside fp16 range (inf*0 -> NaN in the
                            # multiply below if m0 were fp16).
                            m0 = small.tile([128, SKV], BF16, tag="m0", bufs=2)
                            nc.scalar.mul(m0[:], ps[:], 0.125)
                            nc.vector.tensor_tensor(
                                m1[:, s, :], m0[:], A0[:, s, :], OP.mult
                            )
                    E = big.tile([128, NTH, SKV], F16, tag="big", bufs=5)
                    for s in range(NTH):
                        nc.scalar.activation(
                            E[:, s, :], m1[:, s, :], AF.Exp,
                            accum_out=esum[:, s : s + 1],
                        )
                    Ez = big.tile([128, NTH, SKV], F16, tag="big", bufs=5)
                    nc.vector.tensor_tensor(
                        Ez[:].rearrange("p a b -> p (a b)"),
                        E[:].rearrange("p a b -> p (a b)"),
                        b16[:].rearrange("p a b -> p (a b)"),
                        OP.subtract,
                    )
                    nc.vector.tensor_tensor(
                        denom, esum, bsum[:, NTH * half : NTH * half + NTH],
                        OP.subtract,
                    )
                    nc.vector.reciprocal(recip, denom)
                    w16 = big.tile([128, NTH, SKV], F16, tag="bigw", bufs=2)
                    for s in range(NTH):
                        nc.vector.tensor_scalar(
                            w16[:, s, :], Ez[:, s, :],
                            recip[:, s : s + 1], None, OP.mult,
                        )
                    # att_weights out: fp16 -> f32 cast during SWDGE DMA
                    gdma(
                        w_out[h, rows, :].rearrange("(s p) k -> p s k", p=128),
                        w16[:],
                    )

                    # wT via identity matmuls (out = w_tile.T @ I)
                    wT = big.tile([128, NKP, SQH], F16, tag="bigT", bufs=1)
                    for kp in range(NKP):
                        psT = ps_t.tile([128, 512], FP32, tag="ps_t")
                        for s in range(NTH):
                            nc.tensor.matmul(
                                psT[:, 128 * s : 128 * s + 128],
                                w16[:, s, 128 * kp : 128 * kp + 128],
                                I16[:],
                                start=True,
                                stop=True,
                            )
                        if kp % 2 == 0:
                            nc.vector.tensor_copy(wT[:, kp, :], psT[:])
                        else:
                            i_sc = nc.scalar.copy(wT[:, kp, :], psT[:])
                            if h == H - 1:
                                eng_tail["scalar"].append(i_sc)
                    psA = ps_m.tile([128, 512], FP32, tag="ps_m")
                    for kp in range(NKP):
                        nc.tensor.matmul(
                            psA[0:64, 0:SQH],
                            v16[:, kp, 64 * h : 64 * h + 64],
                            wT[:, kp, :],
                            start=(kp == 0),
                            stop=(kp == NKP - 1),
                        )
                    i_at = nc.vector.tensor_copy(
                        attTh[0:64, h, :], psA[0:64, 0:SQH]
                    )
                    if h == H - 1:
                        tail_extra.append(i_at)

                # ---- output projection for this half ---------------------
                for s in range(NTH):
                    sqt = NTH * half + s
                    psO = ps_m.tile([128, 512], FP32, tag="ps_m")
                    for h in range(H):
                        nc.tensor.matmul(
                            psO[:, 0:256],
                            attTh[0:64, h, 128 * s : 128 * s + 128],
                            Wo16[0:64, h, 0:256],
                            start=(h == 0),
                            stop=False,
                        )
                    i_mm = nc.tensor.matmul(
                        psO[:, 0:256], ones16[0:1, 0:128], bo16[0:1, :],
                        start=False, stop=True,
                    )
                    if half == NHALF - 1 and s >= NTH - 2:
                        eng_tail["tensor"].append(i_mm)
                    outt = small.tile([128, DOUT], FP32, tag="outsb")
                    i_oc = nc.vector.tensor_copy(outt[:], psO[:, 0:256])
                    if half == NHALF - 1 and s >= NTH - 2:
                        tail_extra.append(i_oc)
                        eng_tail["vector"].append(i_oc)
                    sdma(out_d[128 * sqt : 128 * sqt + 128, :], outt[:])

            # ---- sync collapse (1 wait per CTRL instruction limit) --------
            deps = (
                [i for lst in eng_tail.values() for i in lst]
                + gp_dmas[-10:]
                + sync_dmas[-12:]
                + tail_extra
            )
            for dep in deps:
                nop = nc.sync.nop()
                add_dep_helper(nop.ins, dep.ins, sync=True, reason="collapse")

    _split_multiwaits(nc)
    return nc


def _split_multiwaits(nc: bass.Bass) -> None:
    """This walrus build allows only ONE sync wait per instruction on the
    CTRL / DMA / TensorScalarPtr pseudo paths. Waits are checked by the
    issuing sequencer in program order, so hoisting all-but-one wait onto
    injected same-engine NoOps immediately before the instruction is
    equivalent."""
    ctr = 0
    for fn in nc.m.functions:
        for blk in fn.blocks:
            new_insts = []
            for inst in blk.instructions:
                si = inst.sync_info
                if si is not None and si.on_wait and len(si.on_wait) > 1:
                    waits = list(si.on_wait)
                    for w in waits[:-1]:
                        ctr += 1
                        nop = mybir.InstNoOp(name=f"I-wsplit-{ctr}", ins=[], outs=[])
                        nop.engine = inst.engine
                        nop.sync_info = mybir.SyncInfo(on_wait=[w], on_update=[])
                        nc.inst_map[nop.name] = nop
                        new_insts.append(nop)
                    inst.sync_info = mybir.SyncInfo(
                        on_wait=[waits[-1]], on_update=list(si.on_update)
                    )
                new_insts.append(inst)
            blk.instructions = new_insts


_NC_CACHE: list = []


def _get_nc() -> bass.Bass:
    if not _NC_CACHE:
        _NC_CACHE.append(_build())
    return _NC_CACHE[0]


def kernel(**inputs) -> tuple:
    from concourse.bass_utils import run_bass_kernel_spmd

    nc = _get_nc()
    f32 = np.float32
    Q = np.ascontiguousarray(np.asarray(inputs["Q"], f32))
    K = np.ascontiguousarray(np.asarray(inputs["K"], f32))
    V = np.ascontiguousarray(np.asarray(inputs["V"], f32))
    aff = np.ascontiguousarray(np.asarray(inputs["q_aff_mask"], f32))
    binm = np.ascontiguousarray(np.asarray(inputs["q_binary_mask"])).view(np.uint8)
    kvm = np.ascontiguousarray(np.asarray(inputs["kv_mask"])).view(np.uint8)
    reps = {
        "Wq": np.ascontiguousarray(np.asarray(inputs["Wq"], f32)),
        "Wk": np.ascontiguousarray(np.asarray(inputs["Wk"], f32)),
        "Wv": np.ascontiguousarray(np.asarray(inputs["Wv"], f32)),
        "Wo": np.ascontiguousarray(np.asarray(inputs["Wo"], f32)),
        "bq": np.asarray(inputs["bq"], f32).reshape(1, DQ),
        "bk": np.asarray(inputs["bk"], f32).reshape(1, DQ),
        "bv": np.asarray(inputs["bv"], f32).reshape(1, DQ),
        "bo": np.asarray(inputs["bo"], f32).reshape(1, DOUT),
    }
    in_maps = []
    for c in range(8):
        b, half = c // 2, c % 2
        sl = slice(half * SQC, (half + 1) * SQC)
        in_maps.append(
            {
                "Qh": Q[b, sl],
                "Kf": K[b],
                "Vf": V[b],
                "aff": aff[b, sl],
                "binm": binm[b, sl],
                "kvm": kvm[b].reshape(1, SKV),
                **reps,
            }
        )
    global _last_in_maps
    _last_in_maps = in_maps
    res = run_bass_kernel_spmd(nc, in_maps, core_ids=list(range(8)))
    out = np.zeros((B, SQ, DOUT), f32)
    att_w = np.zeros((B, H, SQ, SKV), f32)
    for c in range(8):
        b, half = c // 2, c % 2
        sl = slice(half * SQC, (half + 1) * SQC)
        out[b, sl] = res.results[c]["out"]
        att_w[b, :, sl, :] = res.results[c]["w"]
    return out, att_w


if __name__ == "__main__":
    nc = _build()
    print(f"build OK: {len(nc.inst_map)} instructions")
